# revision 1
# baseline (speedup 1.0000x reference)
"""CuPyLinear (sparse CSR y = x @ W.T) Trainium2 kernel, v2 (CSC scatter).

Problem shapes (hardcoded per spec):
  x       [512, 2048] f32
  data    [262144]    f32   (2048 rows x 128 nnz/row, uniform)
  indices [262144]    i32   (sorted per row, duplicates sum)
  indptr  [2049]      i32   (= arange*128, uniform -> unused on device)
  out y   [512, 2048] f32

Sharding: replicate x, shard the 2048 output rows across 8 cores
(256 rows each = 2 row-tiles of 128).

v2 design (vs v1's scatter-W + PE-transpose + PSUM copies):
  Build W.T directly in the matmul lhsT layout with gpsimd local_scatter:
  partition p holds columns {c : c % 128 == p}; the free index of (c, r)
  within row-tile rt is ct*128 + rloc (ct = c // 128). Host precomputes,
  from `indices` only (pure index/layout prep): a permutation of the nnz
  into per-(partition, piece) blocks sorted by target position, the
  duplicate-run `eq` flags, and int16 scatter indices (-1 on all but the
  last of each duplicate run and on pads). Device does all value math:
  a fp16 tensor_tensor_scan sums duplicate runs (fp32 internal state),
  local_scatter densifies W.T, PE contracts y.T[rt] = W.T^T @ x.T in
  fp16 (f32 PSUM), Act+DVE copy halves to SBUF fp16, DMA out.
  PE p-state is ramped with identity transposes built from an on-device
  iota (no DMA dependency), so real matmuls run at full clock.
"""

import os
import sys

sys.path.insert(0, "/opt/trn_rl_repo")

from contextlib import ExitStack

import ml_dtypes
import numpy as np

import concourse.bass as bass
import concourse.tile as tile
from concourse import bacc, mybir
from concourse.bass_utils import run_bass_kernel_spmd

P = 128          # partitions
OUT = 2048       # out features (rows of sparse W)
IN = 2048        # in features (cols of sparse W)
N = 512          # tokens
J = 128          # nnz per row (uniform)
NCORES = 8
R_PER_CORE = OUT // NCORES   # 256
RT = R_PER_CORE // P         # 2 row-tiles per core
CT = IN // P                 # 16 contraction tiles

# Per-rt piece plan: ct ranges of the W.T free axis, fine at the start
# (early matmuls) and at the end (short critical tail). The tail pieces
# run rt0-first so its y copy+DMA hides under rt1's matmuls.
# All three stream plans can be overridden via KCONF (tuning).
import json as _json

_CONF = _json.loads(os.environ.get("KCONF", "{}")) if "KCONF" in os.environ else {}

PIECE_CTS = [tuple(t) for t in _CONF.get(
    "piece_cts", [(0, 4), (4, 8), (8, 12), (12, 14), (14, 16)]
)]
_TAIL_SPLIT = _CONF.get("tail_split", 2)  # trailing ct-ranges emitted rt0-first
PIECES = (
    [(rt, lo, hi) for (lo, hi) in PIECE_CTS[: len(PIECE_CTS) - _TAIL_SPLIT]
     for rt in range(RT)]
    + [(0, lo, hi) for (lo, hi) in PIECE_CTS[-_TAIL_SPLIT:]]
    + [(1, lo, hi) for (lo, hi) in PIECE_CTS[-_TAIL_SPLIT:]]
)
NP_ = len(PIECES)
X_CHUNKS = [tuple(t) for t in _CONF.get(
    "x_chunks", [(0, 4), (4, 8), (8, 12), (12, 14), (14, 16)]
)]
# meta DMA chunks: piece-index ranges (thirds of the piece list)
_M_SPLITS = _CONF.get("m_splits", [0, 2, 6, 10])
META_CHUNKS = [
    (_M_SPLITS[i], _M_SPLITS[i + 1]) for i in range(len(_M_SPLITS) - 1)
]
# dma order: m0 always first, then x chunks with m1.. inserted after the
# x-chunk index given by m_after[i]
_M_AFTER = _CONF.get("m_after", [0, 1])

BF16 = ml_dtypes.bfloat16
F32 = mybir.dt.float32
FP16 = mybir.dt.float16
I16 = mybir.dt.int16

WARMUP = 24      # PE p-state ramp transposes


def build_program(jps):
    """Build + compile the per-core Bass program.

    jps: per-piece num_idxs (even), same order as PIECES.
    """
    nc = bacc.Bacc("TRN2", target_bir_lowering=False, debug=False)

    tot = 3 * sum(jps)
    offs = np.concatenate([[0], np.cumsum([3 * jp for jp in jps])])

    xt_d = nc.dram_tensor("xt", [P, CT, N], FP16, kind="ExternalInput").ap()
    meta_d = nc.dram_tensor("meta", [P, tot], I16, kind="ExternalInput").ap()
    yt_d = nc.dram_tensor("yt", [RT, P, N], FP16, kind="ExternalOutput").ap()

    with tile.TileContext(nc) as tc, ExitStack() as ctx:
        const = ctx.enter_context(tc.tile_pool(name="const", bufs=1))
        mpool = ctx.enter_context(tc.tile_pool(name="meta", bufs=1))
        xpool = ctx.enter_context(tc.tile_pool(name="x", bufs=1))
        spool = ctx.enter_context(tc.tile_pool(name="s", bufs=2))
        wpool = ctx.enter_context(tc.tile_pool(name="w", bufs=1))
        psum_w = ctx.enter_context(tc.tile_pool(name="psum_w", bufs=2, space="PSUM"))
        psum_y = ctx.enter_context(tc.tile_pool(name="psum_y", bufs=2, space="PSUM"))
        ypool = ctx.enter_context(tc.tile_pool(name="y", bufs=2))

        # ---- input DMAs, interleaved so early consumers start early ----
        mtiles = {}
        xtiles = {}
        dma_plan = [("m", 0)]
        for xi in range(len(X_CHUNKS)):
            dma_plan.append(("x", xi))
            for mi, after in enumerate(_M_AFTER):
                if after == xi:
                    dma_plan.append(("m", mi + 1))
        for kind, i in dma_plan:
            if kind == "m":
                plo, phi = META_CHUNKS[i]
                mt = mpool.tile(
                    [P, int(offs[phi] - offs[plo])], I16,
                    name=f"mt{i}", tag=f"mt{i}",
                )
                nc.sync.dma_start(mt[:], meta_d[:, int(offs[plo]) : int(offs[phi])])
                mtiles[i] = (mt, int(offs[plo]))
            else:
                lo, hi = X_CHUNKS[i]
                xt = xpool.tile(
                    [P, hi - lo, N], FP16, name=f"xc{i}", tag=f"xc{i}"
                )
                nc.sync.dma_start(xt[:], xt_d[:, lo:hi, :])
                xtiles[i] = (xt, lo)

        # ---- on-device identity (no DMA dependency) + PE p-state warmup ----
        ii = const.tile([P, P], I16)
        nc.gpsimd.iota(ii[:], [[1, P]], channel_multiplier=-1)
        ident = const.tile([P, P], FP16)
        nc.vector.tensor_scalar(
            ident[:], ii[:], 0.0, None, op0=mybir.AluOpType.is_equal
        )
        # front-load the Act function-table load off the output critical path
        actwarm = const.tile([P, 2], F32)
        nc.vector.memset(actwarm[:, 0:1], 0.0)
        nc.scalar.copy(actwarm[:, 1:2], actwarm[:, 0:1])

        for _ in range(WARMUP):
            warm = psum_w.tile([P, P], FP16, space="PSUM", tag="warm")
            nc.tensor.transpose(warm[:], ident[:], ident[:])

        # ---- per piece: scan (dedupe duplicate runs) then scatter ----
        def meta_ap(k):
            # (v, eq, idx) APs for piece k out of its meta chunk tile
            for ci, (plo, phi) in enumerate(META_CHUNKS):
                if plo <= k < phi:
                    mt, base = mtiles[ci]
                    o = int(offs[k]) - base
                    jp = jps[k]
                    v = mt[:, o : o + jp].bitcast(FP16)
                    eq = mt[:, o + jp : o + 2 * jp].bitcast(FP16)
                    idx = mt[:, o + 2 * jp : o + 3 * jp]
                    return v, eq, idx
            raise AssertionError(k)

        wps = {}
        for k, (rt, lo, hi) in enumerate(PIECES):
            jp = jps[k]
            width = (hi - lo) * P
            v, eq, idx = meta_ap(k)
            s = spool.tile([P, jp], FP16, tag=f"s{k}")
            nc.vector.tensor_tensor_scan(
                s[:], eq, v, 0.0,
                op0=mybir.AluOpType.mult, op1=mybir.AluOpType.add,
            )
            wp = wpool.tile([P, width], FP16, tag=f"wp{k}")
            nc.gpsimd.local_scatter(
                wp[:], s[:], idx, channels=P, num_elems=width, num_idxs=jp
            )
            wps[(rt, lo)] = wp

        # ---- matmuls: y.T[rt] = W.T^T @ x.T, emitted in (ct, rt) order ----
        def piece_of(rt, ct):
            for (prt, lo, hi) in PIECES:
                if prt == rt and lo <= ct < hi:
                    return wps[(rt, lo)], lo
            raise AssertionError((rt, ct))

        def xchunk_of(ct):
            for i, (lo, hi) in enumerate(X_CHUNKS):
                if lo <= ct < hi:
                    xt, base = xtiles[i]
                    return xt, base
            raise AssertionError(ct)

        yps = [
            psum_y.tile([P, N], F32, space="PSUM", tag=f"yp{rt}", name=f"yp{rt}")
            for rt in range(RT)
        ]
        # per piece ct-range, rt0's matmuls run as a block before rt1's
        # (rt0's scatter lands one piece earlier); the tail cts run rt0
        # fully first so rt0's y copy+DMA overlap rt1's last matmuls.
        tail_ct0 = PIECE_CTS[-_TAIL_SPLIT][0]
        mm_order = []
        for lo, hi in PIECE_CTS:
            if lo >= tail_ct0:
                break
            for rt in range(RT):
                mm_order += [(ct, rt) for ct in range(lo, hi)]
        mm_order += [(ct, 0) for ct in range(tail_ct0, CT)]
        mm_order += [(ct, 1) for ct in range(tail_ct0, CT)]
        for ct, rt in mm_order:
            wp, lo = piece_of(rt, ct)
            xt, base = xchunk_of(ct)
            nc.tensor.matmul(
                yps[rt][:],
                wp[:, (ct - lo) * P : (ct - lo + 1) * P],
                xt[:, ct - base, :],
                start=(ct == 0),
                stop=(ct == CT - 1),
            )

        # ---- y out: one full PSUM->SBUF copy per engine (parallel), then
        # fire the prepared descriptors (per-queue).
        # y0's copy+DMA hide under rt1's matmuls. One engine per full copy:
        # slicing one tile across engines serializes on tile-level deps.
        ysb0 = ypool.tile([P, N], FP16, tag="ysb0")
        nc.scalar.copy(ysb0[:], yps[0][:])
        nc.sync.dma_start(yt_d[0], ysb0[:])
        ysb1 = ypool.tile([P, N], FP16, tag="ysb1")
        nc.scalar.copy(ysb1[:], yps[1][:])
        nc.sync.dma_start(yt_d[1], ysb1[:])

    nc.compile()
    return nc


# ---------------------------------------------------------------------------
# Host-side metadata (pure index/layout preprocessing of the CSR pattern)
# ---------------------------------------------------------------------------

_PLAN = None     # (jps, per-core static meta + value scatter positions)


def _build_plan(indices):
    """From `indices` only: per-core permutation + eq/idx metadata."""
    cols = np.asarray(indices).reshape(OUT, J).astype(np.int64)
    nrt = len(PIECE_CTS)
    # piece index within rt by ct
    ct_bounds = np.array([hi for (_, hi) in PIECE_CTS])
    # global piece id by (rt, piece_within_rt) per PIECES order
    gp_of = np.zeros((RT, nrt), np.int64)
    for g, (rt, lo, hi) in enumerate(PIECES):
        gp_of[rt, PIECE_CTS.index((lo, hi))] = g

    cores = []
    counts_all = np.zeros((NCORES, P, NP_), np.int64)
    for core in range(NCORES):
        r0 = core * R_PER_CORE
        sub = cols[r0 : r0 + R_PER_CORE]                     # [256, 128]
        rt = (np.arange(R_PER_CORE) // P)[:, None]
        rloc = (np.arange(R_PER_CORE) % P)[:, None]
        p = sub % P
        ct = sub // P
        free = ct * P + np.broadcast_to(rloc, sub.shape)
        pw = np.searchsorted(ct_bounds, ct, side="right")    # piece within rt
        gp = gp_of[np.broadcast_to(rt, sub.shape), pw]
        src = np.arange(r0 * J, (r0 + R_PER_CORE) * J).reshape(R_PER_CORE, J)

        P_ = p.ravel()
        G_ = gp.ravel()
        F_ = free.ravel()
        S_ = src.ravel()
        order = np.lexsort((F_, G_, P_))
        P_, G_, F_, S_ = P_[order], G_[order], F_[order], S_[order]
        blk = P_ * NP_ + G_
        samerun = (blk[1:] == blk[:-1]) & (F_[1:] == F_[:-1])
        eq = np.concatenate([[False], samerun])
        islast = np.concatenate([~samerun, [True]])
        counts = np.bincount(blk, minlength=P * NP_).reshape(P, NP_)
        counts_all[core] = counts
        cores.append((P_, G_, F_, S_, eq, islast, blk))

    jps = counts_all.max(axis=(0, 1))                        # per piece
    jps = [int(-2 * (-j // 2)) for j in jps]                 # round up to even
    offs = np.concatenate([[0], np.cumsum([3 * jp for jp in jps])])
    tot = int(offs[-1])
    lo_of = {g: lo * P for g, (rt, lo, hi) in enumerate(PIECES)}

    metas = []
    vpos_all = []
    for core in range(NCORES):
        P_, G_, F_, S_, eq, islast, blk = cores[core]
        # slot within block
        blk_start = np.zeros(P * NP_, np.int64)
        first = np.concatenate([[True], blk[1:] != blk[:-1]])
        blk_start[blk[first]] = np.nonzero(first)[0]
        slot = np.arange(len(blk)) - blk_start[blk]
        jp_arr = np.array(jps)[G_]
        o = offs[G_]
        vpos = o + slot
        epos = o + jp_arr + slot
        ipos = o + 2 * jp_arr + slot
        base = np.zeros((P, tot), np.int16)
        # default all idx regions to -1 (pads ignored by local_scatter)
        for g, jp in enumerate(jps):
            base[:, int(offs[g]) + 2 * jp : int(offs[g]) + 3 * jp] = -1
        base[P_, epos] = (
            eq.astype(np.float16).view(np.int16)
        )
        lo_arr = np.array([lo_of[g] for g in range(NP_)])[G_]
        base[P_, ipos] = np.where(islast, F_ - lo_arr, -1).astype(np.int16)
        metas.append(base)
        vpos_all.append((P_, vpos, S_))
    return jps, metas, vpos_all


def _get_plan(indices):
    global _PLAN
    if _PLAN is None:
        _PLAN = _build_plan(indices)
    return _PLAN


_PROGRAM = None
_NEFF_CACHE_DIR = os.path.expanduser("~/.cache/bass_neff")


def _install_neff_disk_cache():
    """Cache the walrus NEFF on disk keyed by BIR hash (the walrus compile
    is ~3.5 min; everything else in a fresh process is seconds)."""
    import hashlib

    import concourse.bass2jax as b2j

    if getattr(b2j.compile_bir_kernel, "_disk_cached", False):
        return
    orig = b2j.compile_bir_kernel

    def cached(bir_json, tmpdir, neff_name="file.neff"):
        canon = bir_json.replace(
            os.path.abspath(__file__).encode(), b"@KERNEL@"
        )
        key = hashlib.sha256(canon).hexdigest()[:32]
        path = os.path.join(_NEFF_CACHE_DIR, f"{key}.neff")
        out = os.path.join(tmpdir, neff_name)
        if os.path.exists(path):
            import shutil

            shutil.copy(path, out)
            return out
        neff_file = orig(bir_json, tmpdir, neff_name=neff_name)
        try:
            os.makedirs(_NEFF_CACHE_DIR, exist_ok=True)
            tmp = path + ".tmp"
            import shutil

            shutil.copy(neff_file, tmp)
            os.replace(tmp, path)
        except OSError:
            pass
        return neff_file

    cached._disk_cached = True
    b2j.compile_bir_kernel = cached


def _get_program(indices=None):
    global _PROGRAM
    if _PROGRAM is None:
        assert indices is not None, "first _get_program call needs indices"
        _install_neff_disk_cache()
        jps, _, _ = _get_plan(indices)
        _PROGRAM = build_program(jps)
    return _PROGRAM


def make_in_maps(x, data, indices):
    """Host-side layout prep + sharding. All value arithmetic (duplicate
    summing, matmul) happens on device; host only permutes/casts."""
    x = np.asarray(x, dtype=np.float32)
    data = np.asarray(data, dtype=np.float32).ravel()

    jps, metas, vpos_all = _get_plan(indices)
    xt = np.ascontiguousarray(
        x.T.reshape(CT, P, N).transpose(1, 0, 2).astype(np.float16)
    )
    d16 = data.astype(np.float16).view(np.int16)

    in_maps = []
    for core in range(NCORES):
        meta = metas[core].copy()
        P_, vpos, S_ = vpos_all[core]
        meta[P_, vpos] = d16[S_]
        in_maps.append({"xt": xt, "meta": meta})
    return in_maps


def kernel(x, data, indices, indptr):
    nc = _get_program(indices)
    in_maps = make_in_maps(x, data, indices)
    res = run_bass_kernel_spmd(nc, in_maps, core_ids=list(range(NCORES)))
    yt = np.concatenate(
        [
            np.asarray(res.results[c]["yt"]).reshape(R_PER_CORE, N)
            for c in range(NCORES)
        ],
        axis=0,
    )  # [OUT, N] == y.T
    return np.ascontiguousarray(yt.T.astype(np.float32))

